# revision 1
# baseline (speedup 1.0000x reference)
"""LoRA QKV fused projection kernel for 8 TRN2 NeuronCores.

Reference computation (T=8192 tokens, HID=4096, D=6144 out, S=8 slots, R=16):
    y = x @ W.T
    a[t,s,i,r] = sum_h x[t,h] * lora_A[s,i,r,h]         (down-proj, all slots)
    a *= onehot(token_to_slot)[t,s] * scaling[s]         (routing gate)
    d[t, :] = concat_i( sum_{s,r} a[t,s,i,r] * B_i[s,:,r] )   (up-proj)
    out = y + d

Sharding (chosen for minimal LDWEIGHTS serialization on the PE):
  * main GEMM: K-split — core c takes hidden dims [c*512, (c+1)*512) and
    computes a full [6144, 8192] fp32 partial of y^T; host reduces the 8
    partials. W k-slice (12.6 MB) streams once; x k-slice (16.8 MB) stays
    resident in SBUF as the moving operand, so each W 128x128 tile is loaded
    into the PE array exactly once per 4 N=512 matmuls.
  * LoRA path: token-split — core c handles tokens [c*1024, (c+1)*1024) with
    the full hidden dim: aT = A @ x_shard^T (PSUM-accumulated over all 32
    k-tiles), multiplied by a host-built gate matrix (onehot * scaling,
    expanded over ranks), then up-projected with B as the stationary operand.
  * all matmuls run in float32r (single-pass fp32 mode, ~1.5e-4 rel err,
    4x the throughput of classic fp32 which needs 2 half-rate passes).

Host pre/post: pure layout rearranges + the 8-way fp32 partial reduce and
final transpose. All routing logic (gate) is exact fp32.
"""

import numpy as np

# problem shape (hardcoded per harness contract)
T = 8192
HID = 4096
Q_SIZE = 4096
KV_SIZE = 1024
D = Q_SIZE + 2 * KV_SIZE  # 6144
S = 8
R = 16
NCORES = 8
P = 128

KC = HID // NCORES        # 512 hidden dims per core (main GEMM K-shard)
KT = KC // P              # 4 k-tiles per core
TC = T // NCORES          # 1024 tokens per core (LoRA shard)
MB = D // P               # 48 output row-blocks of 128
KA = HID // P             # 32 k-tiles for the LoRA down-proj
NT = T // 512             # 16 moving n-tiles of 512 tokens
NG = 4                    # psum groups of 4 tiles (4 banks, double buffered)
NJ = NT // NG             # 4 n-tiles per psum group

_CACHE = {}


def _build_nc():
    import concourse.mybir as mybir
    import concourse.tile as tile
    from concourse import bacc

    dtr = mybir.dt.float32r
    f32 = mybir.dt.float32

    nc = bacc.Bacc(None, target_bir_lowering=False, debug=False)

    # ---- DRAM parameters (per-core shapes; declaration order = binding order)
    x_res_d = nc.declare_dram_parameter("x_res", [P, KT, T], dtr, isOutput=False)
    w_d = nc.declare_dram_parameter("w_t", [MB, P, KT, P], dtr, isOutput=False)
    xl_d = nc.declare_dram_parameter("x_lora", [P, KA, TC], dtr, isOutput=False)
    a_d = nc.declare_dram_parameter("a_t", [P, KA, 3, P], dtr, isOutput=False)
    b_d = nc.declare_dram_parameter("b_t", [P, MB, P], dtr, isOutput=False)
    g_d = nc.declare_dram_parameter("gate", [P, TC], f32, isOutput=False)
    y_d = nc.declare_dram_parameter("y_part", [MB, P, T], f32, isOutput=True)
    d_d = nc.declare_dram_parameter("d_out", [MB, P, TC], f32, isOutput=True)

    with tile.TileContext(nc) as tc:
        with tc.tile_pool(name="xres", bufs=1) as xres_pool, \
             tc.tile_pool(name="big", bufs=3) as big_pool, \
             tc.tile_pool(name="wp", bufs=3) as w_pool, \
             tc.tile_pool(name="ap", bufs=2) as a_pool, \
             tc.tile_pool(name="agp", bufs=1) as ag_pool, \
             tc.tile_pool(name="bp", bufs=3) as b_pool, \
             tc.tile_pool(name="dsp", bufs=3) as ds_pool, \
             tc.tile_pool(name="psum", bufs=8, space="PSUM") as ps_pool:

            # resident moving operand for the main GEMM: [p(k), kt, t]
            x_res = xres_pool.tile([P, KT, T], dtr, tag="xres")
            nc.sync.dma_start(out=x_res[:], in_=x_res_d[:])

            # ---------------- Phase A: LoRA down-proj aT = A @ xl ----------
            # aT[(i,sr), t] accumulated in 6 psum tiles over all 32 k-tiles.
            ps_a = [
                ps_pool.tile([P, 512], f32, tag="ps", name=f"ps_a{i}_{h}")
                for i in range(3) for h in range(2)
            ]
            KCH = 2  # k-tiles per streamed chunk
            for ch in range(KA // KCH):
                xl_t = big_pool.tile([P, KCH, TC], dtr, tag="big", name=f"xl{ch}")
                nc.sync.dma_start(out=xl_t[:], in_=xl_d[:, ch * KCH:(ch + 1) * KCH, :])
                a_t = a_pool.tile([P, KCH, 3, P], dtr, tag="a", name=f"a{ch}")
                nc.sync.dma_start(out=a_t[:], in_=a_d[:, ch * KCH:(ch + 1) * KCH, :, :])
                for kk in range(KCH):
                    first = ch == 0 and kk == 0
                    last = ch == KA // KCH - 1 and kk == KCH - 1
                    for i in range(3):
                        for h in range(2):
                            nc.tensor.matmul(
                                ps_a[i * 2 + h][:],
                                a_t[:, kk, i, :],
                                xl_t[:, kk, h * 512:(h + 1) * 512],
                                start=first, stop=last,
                            )

            # ---------------- Phase B: routing gate ------------------------
            gate_t = ag_pool.tile([P, TC], f32, tag="gate")
            nc.sync.dma_start(out=gate_t[:], in_=g_d[:])
            ag = []
            for i in range(3):
                ag_t = ag_pool.tile([P, TC], dtr, tag=f"ag{i}", name=f"ag{i}")
                for h in range(2):
                    sl = slice(h * 512, (h + 1) * 512)
                    nc.vector.tensor_mul(ag_t[:, sl], ps_a[i * 2 + h][:], gate_t[:, sl])
                ag.append(ag_t)

            # ---------------- Phase C: main GEMM partials -------------------
            # out_part[(mb,dl), t] += W_kc[:, mb].T @ x_kc  for this core's
            # hidden k-shard.  W tile is stationary: 1 LDWEIGHTS per (mb, ng,
            # kt) amortized over NJ=4 N=512 matmuls.
            for mb in range(MB):
                w_t = w_pool.tile([P, KT, P], dtr, tag="w", name=f"w{mb}")
                nc.sync.dma_start(out=w_t[:], in_=w_d[mb])
                for ng in range(NG):
                    pss = [
                        ps_pool.tile([P, 512], f32, tag="ps", name=f"pm{mb}_{ng}_{j}")
                        for j in range(NJ)
                    ]
                    for kk in range(KT):
                        for j in range(NJ):
                            t0 = (ng * NJ + j) * 512
                            nc.tensor.matmul(
                                pss[j][:],
                                w_t[:, kk, :],
                                x_res[:, kk, t0:t0 + 512],
                                start=(kk == 0), stop=(kk == KT - 1),
                            )
                    st = big_pool.tile([P, NJ * 512], f32, tag="big", name=f"st{mb}_{ng}")
                    for j in range(NJ):
                        nc.vector.tensor_copy(st[:, j * 512:(j + 1) * 512], pss[j][:])
                    nc.scalar.dma_start(
                        out=y_d[mb, :, ng * NJ * 512:(ng + 1) * NJ * 512], in_=st[:]
                    )

            # ---------------- Phase D: LoRA up-proj -------------------------
            import concourse.mybir as _mybir
            for mb in range(MB):
                b_t = b_pool.tile([P, P], dtr, tag="b", name=f"b{mb}")
                nc.sync.dma_start(out=b_t[:], in_=b_d[:, mb, :])
                i = 0 if mb < Q_SIZE // P else (1 if mb < (Q_SIZE + KV_SIZE) // P else 2)
                for h in range(2):
                    ps_u = ps_pool.tile([P, 512], f32, tag="ps", name=f"pu{mb}_{h}")
                    nc.tensor.matmul(
                        ps_u[:],
                        b_t[:],
                        ag[i][:, h * 512:(h + 1) * 512],
                        start=True, stop=True,
                    )
                    dst = ds_pool.tile([P, 512], f32, tag="dst", name=f"d{mb}_{h}")
                    nc.vector.tensor_copy(dst[:], ps_u[:])
                    nc.scalar.dma_start(out=d_d[mb, :, h * 512:(h + 1) * 512], in_=dst[:])

    nc.compile()
    return nc


def _get_nc():
    if "nc" not in _CACHE:
        _CACHE["nc"] = _build_nc()
    return _CACHE["nc"]


def _prep_in_maps(x, W, lora_A, lora_B_q, lora_B_k, lora_B_v, scaling, token_to_slot):
    f = np.float32
    x = np.ascontiguousarray(x, dtype=f)
    W = np.ascontiguousarray(W, dtype=f)

    # main GEMM moving operand: [c, p, kt, t]  (h = c*512 + kt*128 + p)
    x_res = np.ascontiguousarray(
        x.reshape(T, NCORES, KT, P).transpose(1, 3, 2, 0))
    # main GEMM stationary: [c, mb, p, kt, dl]  (d = mb*128 + dl)
    w_t = np.ascontiguousarray(
        W.reshape(MB, P, NCORES, KT, P).transpose(2, 0, 4, 3, 1))
    # LoRA down-proj moving operand: [c, p, ka, tl]  (t = c*1024 + tl)
    x_lora = np.ascontiguousarray(
        x.reshape(NCORES, TC, KA, P).transpose(0, 3, 2, 1))
    # LoRA A stationary: [p, ka, i, (s r)]
    a_t = np.ascontiguousarray(
        np.asarray(lora_A, dtype=f).reshape(S, 3, R, KA, P).transpose(4, 3, 1, 0, 2)
        .reshape(P, KA, 3, S * R))
    # LoRA B stationary: [(s r), mb, dl]
    bq = np.asarray(lora_B_q, dtype=f).transpose(0, 2, 1).reshape(S * R, Q_SIZE)
    bk = np.asarray(lora_B_k, dtype=f).transpose(0, 2, 1).reshape(S * R, KV_SIZE)
    bv = np.asarray(lora_B_v, dtype=f).transpose(0, 2, 1).reshape(S * R, KV_SIZE)
    b_t = np.ascontiguousarray(
        np.concatenate([bq, bk, bv], axis=1).reshape(S * R, MB, P))
    # routing gate, expanded over ranks: [c, (s r), tl]
    slot = np.asarray(token_to_slot).reshape(NCORES, TC)
    g = (slot[:, None, :] == np.arange(S, dtype=slot.dtype)[None, :, None])
    g = g.astype(f) * np.asarray(scaling, dtype=f)[None, :, None]
    gate = np.ascontiguousarray(np.repeat(g, R, axis=1))

    in_maps = []
    for c in range(NCORES):
        in_maps.append({
            "x_res": x_res[c],
            "w_t": w_t[c],
            "x_lora": x_lora[c],
            "a_t": a_t,
            "b_t": b_t,
            "gate": gate[c],
        })
    return in_maps


def _assemble(results):
    # reduce the 8 main-GEMM partials: [mb, dl, t] fp32
    acc = results[0]["y_part"].copy()
    for c in range(1, NCORES):
        acc += results[c]["y_part"]
    yT = acc.reshape(D, T)
    # add each core's LoRA delta into its token columns
    for c in range(NCORES):
        yT[:, c * TC:(c + 1) * TC] += results[c]["d_out"].reshape(D, TC)
    return np.ascontiguousarray(yT.T)


def _run(inputs, trace=False):
    from concourse.bass_utils import run_bass_kernel_spmd
    nc = _get_nc()
    in_maps = _prep_in_maps(**inputs)
    res = run_bass_kernel_spmd(
        nc, in_maps, core_ids=list(range(NCORES)), trace=trace)
    return res


def kernel(**inputs) -> np.ndarray:
    res = _run(inputs, trace=False)
    return _assemble(res.results)


if __name__ == "__main__":
    rng = np.random.default_rng(0)
    ins = {
        "x": rng.standard_normal((T, HID)).astype(np.float32),
        "W": (rng.standard_normal((D, HID)) * 0.02).astype(np.float32),
        "lora_A": (rng.standard_normal((S, 3, R, HID)) * 0.02).astype(np.float32),
        "lora_B_q": (rng.standard_normal((S, Q_SIZE, R)) * 0.02).astype(np.float32),
        "lora_B_k": (rng.standard_normal((S, KV_SIZE, R)) * 0.02).astype(np.float32),
        "lora_B_v": (rng.standard_normal((S, KV_SIZE, R)) * 0.02).astype(np.float32),
        "scaling": rng.uniform(0.5, 2.0, S).astype(np.float32),
        "token_to_slot": rng.integers(0, S, T).astype(np.int32),
    }
    out = kernel(**ins)
    print("out", out.shape, out.dtype)



# revision 2
# speedup vs baseline: 1.2189x; 1.2189x over previous
"""LoRA QKV fused projection kernel for 8 TRN2 NeuronCores.

Reference computation (T=8192 tokens, HID=4096, D=6144 out, S=8 slots, R=16):
    y = x @ W.T
    a[t,s,i,r] = sum_h x[t,h] * lora_A[s,i,r,h]         (down-proj, all slots)
    a *= onehot(token_to_slot)[t,s] * scaling[s]         (routing gate)
    d[t, :] = concat_i( sum_{s,r} a[t,s,i,r] * B_i[s,:,r] )   (up-proj)
    out = y + d

Sharding: pure token-DP — core c owns tokens [c*1024, (c+1)*1024) with the
full hidden and output dims. Its x shard (8.4 MB bf16) is loaded to SBUF once
and serves as the moving operand for BOTH the LoRA down-proj and the main
GEMM. W streams through once (50 MB bf16 per core).

All matmul inputs are bf16 (host-converted): bf16 enables the fast weight
load path (FWL, 4 elem/cycle — fp32/fp32r cannot FWL), so the per-matmul
stationary reload (~27 ns) hides under the previous matmul's 512-column
stream inside the PE's 64-deep reorder window. PSUM accumulates in fp32, so
the only precision loss is the one-time bf16 input rounding (~1.5e-3 rel).

The LoRA up-proj is fused into the main GEMM's PSUM accumulation: per output
row-block mb, after the 32 K-tile matmuls, one extra matmul with B as the
stationary operand (contracting the 128 (slot,rank) pairs of the gated
down-proj activations) lands the LoRA delta in the same PSUM bank
(start=False). A single drain then emits final y rows — no partials, no
host-side reduce.
"""

import numpy as np

# problem shape (hardcoded per harness contract)
T = 8192
HID = 4096
Q_SIZE = 4096
KV_SIZE = 1024
D = Q_SIZE + 2 * KV_SIZE  # 6144
S = 8
R = 16
NCORES = 8
P = 128

TC = T // NCORES          # 1024 tokens per core
KA = HID // P             # 32 k-tiles
MB = D // P               # 48 output row-blocks of 128
NH = TC // 512            # 2 moving n-halves of 512 tokens
CH = 4                    # k-tiles per down-proj/x-load chunk
NCH = KA // CH            # 8 chunks

_CACHE = {}


def _build_nc():
    import concourse.mybir as mybir
    import concourse.tile as tile
    from concourse import bacc

    bf16 = mybir.dt.bfloat16
    f32 = mybir.dt.float32
    COPY = mybir.ActivationFunctionType.Copy

    nc = bacc.Bacc(None, target_bir_lowering=False, debug=False)

    # ---- DRAM parameters (per-core shapes; declaration order = binding order)
    x_d = nc.declare_dram_parameter("x_t", [P, KA, TC], bf16, isOutput=False)
    a_d = nc.declare_dram_parameter("a_t", [P, KA, 3, P], bf16, isOutput=False)
    w_d = nc.declare_dram_parameter("w_t", [MB, P, KA, P], bf16, isOutput=False)
    b_d = nc.declare_dram_parameter("b_t", [P, MB, P], bf16, isOutput=False)
    g_d = nc.declare_dram_parameter("gate", [P, TC], f32, isOutput=False)
    y_d = nc.declare_dram_parameter("y", [MB, P, TC], f32, isOutput=True)

    with tile.TileContext(nc) as tc:
        with tc.tile_pool(name="xres", bufs=1) as xres_pool, \
             tc.tile_pool(name="wp", bufs=3) as w_pool, \
             tc.tile_pool(name="ap", bufs=2) as a_pool, \
             tc.tile_pool(name="agp", bufs=1) as ag_pool, \
             tc.tile_pool(name="bp", bufs=1) as b_pool, \
             tc.tile_pool(name="stp", bufs=4) as st_pool, \
             tc.tile_pool(name="psum", bufs=8, space="PSUM") as ps_pool:

            # resident moving operand: x shard [p(k), kt, t], filled chunkwise
            x_res = xres_pool.tile([P, KA, TC], bf16, tag="xres")

            b_t = b_pool.tile([P, MB, P], bf16, tag="b")
            nc.sync.dma_start(out=b_t[:], in_=b_d[:])
            gate_t = ag_pool.tile([P, TC], f32, tag="gate")
            nc.sync.dma_start(out=gate_t[:], in_=g_d[:])

            # ---------------- Phase A: LoRA down-proj aT = A @ x ------------
            # aT[(i,sr), t] accumulated in 6 psum banks over all 32 k-tiles,
            # chasing the chunked x/A loads.
            ps_a = [
                ps_pool.tile([P, 512], f32, tag="ps", name=f"ps_a{i}_{h}")
                for i in range(3) for h in range(2)
            ]
            for ch in range(NCH):
                ksl = slice(ch * CH, (ch + 1) * CH)
                nc.sync.dma_start(out=x_res[:, ksl, :], in_=x_d[:, ksl, :])
                a_t = a_pool.tile([P, CH, 3, P], bf16, tag="a", name=f"a{ch}")
                nc.sync.dma_start(out=a_t[:], in_=a_d[:, ksl, :, :])
                for kk in range(CH):
                    first = ch == 0 and kk == 0
                    last = ch == NCH - 1 and kk == CH - 1
                    for i in range(3):
                        for h in range(2):
                            nc.tensor.matmul(
                                ps_a[i * 2 + h][:],
                                a_t[:, kk, i, :],
                                x_res[:, ch * CH + kk, h * 512:(h + 1) * 512],
                                start=first, stop=last,
                            )

            # ---------------- Phase B: routing gate -------------------------
            ag = []
            for i in range(3):
                ag_t = ag_pool.tile([P, TC], bf16, tag=f"ag{i}", name=f"ag{i}")
                for h in range(2):
                    sl = slice(h * 512, (h + 1) * 512)
                    nc.vector.tensor_mul(ag_t[:, sl], ps_a[i * 2 + h][:], gate_t[:, sl])
                ag.append(ag_t)

            # ---------------- Phase C: main GEMM + fused LoRA up-proj -------
            for mb in range(MB):
                w_t = w_pool.tile([P, KA, P], bf16, tag="w", name=f"w{mb}")
                nc.sync.dma_start(out=w_t[:], in_=w_d[mb])
                i = 0 if mb < Q_SIZE // P else (1 if mb < (Q_SIZE + KV_SIZE) // P else 2)
                pss = [
                    ps_pool.tile([P, 512], f32, tag="ps", name=f"pm{mb}_{j}")
                    for j in range(NH)
                ]
                for kt in range(KA):
                    for j in range(NH):
                        nc.tensor.matmul(
                            pss[j][:],
                            w_t[:, kt, :],
                            x_res[:, kt, j * 512:(j + 1) * 512],
                            start=(kt == 0), stop=False,
                        )
                for j in range(NH):
                    nc.tensor.matmul(
                        pss[j][:],
                        b_t[:, mb, :],
                        ag[i][:, j * 512:(j + 1) * 512],
                        start=False, stop=True,
                    )
                st = st_pool.tile([P, TC], f32, tag="st", name=f"st{mb}")
                nc.vector.tensor_copy(st[:, 0:512], pss[0][:])
                nc.scalar.activation(st[:, 512:1024], pss[1][:], COPY)
                nc.scalar.dma_start(out=y_d[mb], in_=st[:])

    nc.compile()
    return nc


def _get_nc():
    if "nc" not in _CACHE:
        _CACHE["nc"] = _build_nc()
    return _CACHE["nc"]


def _prep_in_maps(x, W, lora_A, lora_B_q, lora_B_k, lora_B_v, scaling, token_to_slot):
    import ml_dtypes
    bf = ml_dtypes.bfloat16
    f = np.float32
    x = np.asarray(x, dtype=f)
    W = np.asarray(W, dtype=f)

    # x moving operand, token-sharded: [c, p(k), kt, tl]  (t = c*1024 + tl)
    x_t = np.ascontiguousarray(
        x.reshape(NCORES, TC, KA, P).transpose(0, 3, 2, 1).astype(bf))
    # main GEMM stationary (replicated): [mb, p(k), kt, dl]  (d = mb*128 + dl)
    w_t = np.ascontiguousarray(
        W.reshape(MB, P, KA, P).transpose(0, 3, 2, 1).astype(bf))
    # LoRA A stationary: [p(k), kt, i, (s r)]
    a_t = np.ascontiguousarray(
        np.asarray(lora_A, dtype=f).reshape(S, 3, R, KA, P).transpose(4, 3, 1, 0, 2)
        .reshape(P, KA, 3, S * R).astype(bf))
    # LoRA B stationary: [(s r), mb, dl]
    bq = np.asarray(lora_B_q, dtype=f).transpose(0, 2, 1).reshape(S * R, Q_SIZE)
    bk = np.asarray(lora_B_k, dtype=f).transpose(0, 2, 1).reshape(S * R, KV_SIZE)
    bv = np.asarray(lora_B_v, dtype=f).transpose(0, 2, 1).reshape(S * R, KV_SIZE)
    b_t = np.ascontiguousarray(
        np.concatenate([bq, bk, bv], axis=1).reshape(S * R, MB, P).astype(bf))
    # routing gate, expanded over ranks: [c, (s r), tl]
    slot = np.asarray(token_to_slot).reshape(NCORES, TC)
    g = (slot[:, None, :] == np.arange(S, dtype=slot.dtype)[None, :, None])
    g = g.astype(f) * np.asarray(scaling, dtype=f)[None, :, None]
    gate = np.ascontiguousarray(np.repeat(g, R, axis=1))

    in_maps = []
    for c in range(NCORES):
        in_maps.append({
            "x_t": x_t[c],
            "a_t": a_t,
            "w_t": w_t,
            "b_t": b_t,
            "gate": gate[c],
        })
    return in_maps


def _assemble(results):
    # y[c] is [mb, dl, tl] fp32 — final values for core c's token shard
    return np.ascontiguousarray(np.concatenate(
        [results[c]["y"].reshape(D, TC).T for c in range(NCORES)], axis=0))


def _run(inputs, trace=False):
    from concourse.bass_utils import run_bass_kernel_spmd
    nc = _get_nc()
    in_maps = _prep_in_maps(**inputs)
    res = run_bass_kernel_spmd(
        nc, in_maps, core_ids=list(range(NCORES)), trace=trace)
    return res


def kernel(**inputs) -> np.ndarray:
    res = _run(inputs, trace=False)
    return _assemble(res.results)


if __name__ == "__main__":
    rng = np.random.default_rng(0)
    ins = {
        "x": rng.standard_normal((T, HID)).astype(np.float32),
        "W": (rng.standard_normal((D, HID)) * 0.02).astype(np.float32),
        "lora_A": (rng.standard_normal((S, 3, R, HID)) * 0.02).astype(np.float32),
        "lora_B_q": (rng.standard_normal((S, Q_SIZE, R)) * 0.02).astype(np.float32),
        "lora_B_k": (rng.standard_normal((S, KV_SIZE, R)) * 0.02).astype(np.float32),
        "lora_B_v": (rng.standard_normal((S, KV_SIZE, R)) * 0.02).astype(np.float32),
        "scaling": rng.uniform(0.5, 2.0, S).astype(np.float32),
        "token_to_slot": rng.integers(0, S, T).astype(np.int32),
    }
    out = kernel(**ins)
    print("out", out.shape, out.dtype)


# revision 7
# speedup vs baseline: 1.4616x; 1.1991x over previous
"""LoRA QKV fused projection kernel for 8 TRN2 NeuronCores.

Reference computation (T=8192 tokens, HID=4096, D=6144 out, S=8 slots, R=16):
    y = x @ W.T
    a[t,s,i,r] = sum_h x[t,h] * lora_A[s,i,r,h]         (down-proj, all slots)
    a *= onehot(token_to_slot)[t,s] * scaling[s]         (routing gate)
    d[t, :] = concat_i( sum_{s,r} a[t,s,i,r] * B_i[s,:,r] )   (up-proj)
    out = y + d

Sharding: pure token-DP — core c owns tokens [c*1024, (c+1)*1024) with the
full hidden and output dims. Its x shard (8.4 MB bf16) is loaded to SBUF once
and serves as the moving operand for BOTH the LoRA down-proj and the main
GEMM. W streams through once (50 MB bf16 per core).

All matmul inputs are bf16 (host-converted): bf16 enables the fast weight
load path (FWL, 4 elem/cycle — fp32/fp32r cannot FWL), so the per-matmul
stationary reload (~27 ns) hides under the previous matmul's 512-column
stream inside the PE's 64-deep reorder window. PSUM accumulates in fp32, so
the only precision loss is the one-time bf16 input rounding (~1.5e-3 rel).

The LoRA up-proj is fused into the main GEMM's PSUM accumulation: per output
row-block mb, after the 32 K-tile matmuls, one extra matmul with B as the
stationary operand (contracting the 128 (slot,rank) pairs of the gated
down-proj activations) lands the LoRA delta in the same PSUM bank
(start=False). A single drain then emits final y rows — no partials, no
host-side reduce.
"""

import numpy as np

# problem shape (hardcoded per harness contract)
T = 8192
HID = 4096
Q_SIZE = 4096
KV_SIZE = 1024
D = Q_SIZE + 2 * KV_SIZE  # 6144
S = 8
R = 16
NCORES = 8
P = 128

TC = T // NCORES          # 1024 tokens per core
KA = HID // P             # 32 k-tiles
MB = D // P               # 48 output row-blocks of 128
NH = TC // 512            # 2 moving n-halves of 512 tokens
CH = 2                    # k-tiles per down-proj/x-load chunk
NCH = KA // CH            # 16 chunks

_CACHE = {}


def _build_nc():
    import concourse.mybir as mybir
    import concourse.tile as tile
    from concourse import bacc

    bf16 = mybir.dt.bfloat16
    f32 = mybir.dt.float32
    COPY = mybir.ActivationFunctionType.Copy

    nc = bacc.Bacc(None, target_bir_lowering=False, debug=False)

    # ---- DRAM parameters (per-core shapes; declaration order = binding order)
    x_d = nc.declare_dram_parameter("x_t", [P, KA, TC], bf16, isOutput=False)
    a_d = nc.declare_dram_parameter("a_t", [P, KA, 3, P], bf16, isOutput=False)
    w_d = nc.declare_dram_parameter("w_t", [MB, P, KA, P], bf16, isOutput=False)
    b_d = nc.declare_dram_parameter("b_t", [P, MB, P], bf16, isOutput=False)
    g_d = nc.declare_dram_parameter("gate", [P, TC], f32, isOutput=False)
    y_d = nc.declare_dram_parameter("y", [MB, P, TC], f32, isOutput=True)

    with tile.TileContext(nc) as tc:
        with tc.tile_pool(name="xres", bufs=1) as xres_pool, \
             tc.tile_pool(name="wp", bufs=3) as w_pool, \
             tc.tile_pool(name="ap", bufs=16) as a_pool, \
             tc.tile_pool(name="agp", bufs=1) as ag_pool, \
             tc.tile_pool(name="bp", bufs=1) as b_pool, \
             tc.tile_pool(name="stp", bufs=4) as st_pool, \
             tc.tile_pool(name="psum", bufs=8, space="PSUM") as ps_pool:

            # resident moving operand: x shard [p(k), kt, t], filled chunkwise
            x_res = xres_pool.tile([P, KA, TC], bf16, tag="xres")

            # ---------------- Phase A: LoRA down-proj aT = A @ x ------------
            # aT[(i,sr), t] accumulated in 6 psum banks over all 32 k-tiles,
            # chasing the chunked x/A loads on the SP (HWDGE) queue. The
            # B/gate loads ride the ACT queue (idle until outputs start), so
            # the first matmul starts after ~0.7 MB instead of ~3.5 MB.
            ps_a = [
                ps_pool.tile([P, 512], f32, tag="ps", name=f"ps_a{i}_{h}")
                for i in range(3) for h in range(2)
            ]
            a_tiles = []
            for ch in range(NCH):
                ksl = slice(ch * CH, (ch + 1) * CH)
                nc.sync.dma_start(out=x_res[:, ksl, :], in_=x_d[:, ksl, :])
                a_t = a_pool.tile([P, CH, 3, P], bf16, tag="a", name=f"a{ch}")
                nc.sync.dma_start(out=a_t[:], in_=a_d[:, ksl, :, :])
                a_tiles.append(a_t)
            b_t = b_pool.tile([P, MB, P], bf16, tag="b")
            nc.scalar.dma_start(out=b_t[:], in_=b_d[:])
            gate_t = ag_pool.tile([P, TC], f32, tag="gate")
            nc.scalar.dma_start(out=gate_t[:], in_=g_d[:])
            for ch in range(NCH):
                a_t = a_tiles[ch]
                for kk in range(CH):
                    first = ch == 0 and kk == 0
                    last = ch == NCH - 1 and kk == CH - 1
                    for i in range(3):
                        for h in range(2):
                            nc.tensor.matmul(
                                ps_a[i * 2 + h][:],
                                a_t[:, kk, i, :],
                                x_res[:, ch * CH + kk, h * 512:(h + 1) * 512],
                                start=first, stop=last,
                            )

            # ---------------- Phase B: routing gate -------------------------
            ag = []
            for i in range(3):
                ag_t = ag_pool.tile([P, TC], bf16, tag=f"ag{i}", name=f"ag{i}")
                for h in range(2):
                    sl = slice(h * 512, (h + 1) * 512)
                    nc.vector.tensor_mul(ag_t[:, sl], ps_a[i * 2 + h][:], gate_t[:, sl])
                ag.append(ag_t)

            # ---------------- Phase C: main GEMM + fused LoRA up-proj -------
            for mb in range(MB):
                w_t = w_pool.tile([P, KA, P], bf16, tag="w", name=f"w{mb}")
                nc.sync.dma_start(out=w_t[:], in_=w_d[mb])
                i = 0 if mb < Q_SIZE // P else (1 if mb < (Q_SIZE + KV_SIZE) // P else 2)
                pss = [
                    ps_pool.tile([P, 512], f32, tag="ps", name=f"pm{mb}_{j}")
                    for j in range(NH)
                ]
                for kt in range(KA):
                    for j in range(NH):
                        nc.tensor.matmul(
                            pss[j][:],
                            w_t[:, kt, :],
                            x_res[:, kt, j * 512:(j + 1) * 512],
                            start=(kt == 0), stop=False,
                        )
                for j in range(NH):
                    nc.tensor.matmul(
                        pss[j][:],
                        b_t[:, mb, :],
                        ag[i][:, j * 512:(j + 1) * 512],
                        start=False, stop=True,
                    )
                st = st_pool.tile([P, TC], f32, tag="st", name=f"st{mb}")
                nc.vector.tensor_copy(st[:, 0:512], pss[0][:])
                nc.scalar.activation(st[:, 512:1024], pss[1][:], COPY)
                nc.scalar.dma_start(out=y_d[mb, :, 0:512], in_=st[:, 0:512])
                nc.scalar.dma_start(out=y_d[mb, :, 512:1024], in_=st[:, 512:1024])

    nc.compile()
    return nc


def _get_nc():
    if "nc" not in _CACHE:
        _CACHE["nc"] = _build_nc()
    return _CACHE["nc"]


def _prep_in_maps(x, W, lora_A, lora_B_q, lora_B_k, lora_B_v, scaling, token_to_slot):
    import ml_dtypes
    bf = ml_dtypes.bfloat16
    f = np.float32
    x = np.asarray(x, dtype=f)
    W = np.asarray(W, dtype=f)

    # x moving operand, token-sharded: [c, p(k), kt, tl]  (t = c*1024 + tl)
    x_t = np.ascontiguousarray(
        x.reshape(NCORES, TC, KA, P).transpose(0, 3, 2, 1).astype(bf))
    # main GEMM stationary (replicated): [mb, p(k), kt, dl]  (d = mb*128 + dl)
    w_t = np.ascontiguousarray(
        W.reshape(MB, P, KA, P).transpose(0, 3, 2, 1).astype(bf))
    # LoRA A stationary: [p(k), kt, i, (s r)]
    a_t = np.ascontiguousarray(
        np.asarray(lora_A, dtype=f).reshape(S, 3, R, KA, P).transpose(4, 3, 1, 0, 2)
        .reshape(P, KA, 3, S * R).astype(bf))
    # LoRA B stationary: [(s r), mb, dl]
    bq = np.asarray(lora_B_q, dtype=f).transpose(0, 2, 1).reshape(S * R, Q_SIZE)
    bk = np.asarray(lora_B_k, dtype=f).transpose(0, 2, 1).reshape(S * R, KV_SIZE)
    bv = np.asarray(lora_B_v, dtype=f).transpose(0, 2, 1).reshape(S * R, KV_SIZE)
    b_t = np.ascontiguousarray(
        np.concatenate([bq, bk, bv], axis=1).reshape(S * R, MB, P).astype(bf))
    # routing gate, expanded over ranks: [c, (s r), tl]
    slot = np.asarray(token_to_slot).reshape(NCORES, TC)
    g = (slot[:, None, :] == np.arange(S, dtype=slot.dtype)[None, :, None])
    g = g.astype(f) * np.asarray(scaling, dtype=f)[None, :, None]
    gate = np.ascontiguousarray(np.repeat(g, R, axis=1))

    in_maps = []
    for c in range(NCORES):
        in_maps.append({
            "x_t": x_t[c],
            "a_t": a_t,
            "w_t": w_t,
            "b_t": b_t,
            "gate": gate[c],
        })
    return in_maps


def _assemble(results):
    # y[c] is [mb, dl, tl] fp32 — final values for core c's token shard
    return np.ascontiguousarray(np.concatenate(
        [results[c]["y"].reshape(D, TC).T for c in range(NCORES)], axis=0))


def _run(inputs, trace=False):
    from concourse.bass_utils import run_bass_kernel_spmd
    nc = _get_nc()
    in_maps = _prep_in_maps(**inputs)
    res = run_bass_kernel_spmd(
        nc, in_maps, core_ids=list(range(NCORES)), trace=trace)
    return res


def kernel(**inputs) -> np.ndarray:
    res = _run(inputs, trace=False)
    return _assemble(res.results)


if __name__ == "__main__":
    rng = np.random.default_rng(0)
    ins = {
        "x": rng.standard_normal((T, HID)).astype(np.float32),
        "W": (rng.standard_normal((D, HID)) * 0.02).astype(np.float32),
        "lora_A": (rng.standard_normal((S, 3, R, HID)) * 0.02).astype(np.float32),
        "lora_B_q": (rng.standard_normal((S, Q_SIZE, R)) * 0.02).astype(np.float32),
        "lora_B_k": (rng.standard_normal((S, KV_SIZE, R)) * 0.02).astype(np.float32),
        "lora_B_v": (rng.standard_normal((S, KV_SIZE, R)) * 0.02).astype(np.float32),
        "scaling": rng.uniform(0.5, 2.0, S).astype(np.float32),
        "token_to_slot": rng.integers(0, S, T).astype(np.int32),
    }
    out = kernel(**ins)
    print("out", out.shape, out.dtype)


# revision 10
# speedup vs baseline: 1.4662x; 1.0032x over previous
"""LoRA QKV fused projection kernel for 8 TRN2 NeuronCores.

Reference computation (T=8192 tokens, HID=4096, D=6144 out, S=8 slots, R=16):
    y = x @ W.T
    a[t,s,i,r] = sum_h x[t,h] * lora_A[s,i,r,h]         (down-proj, all slots)
    a *= onehot(token_to_slot)[t,s] * scaling[s]         (routing gate)
    d[t, :] = concat_i( sum_{s,r} a[t,s,i,r] * B_i[s,:,r] )   (up-proj)
    out = y + d

Sharding: pure token-DP — core c owns tokens [c*1024, (c+1)*1024) with the
full hidden and output dims. Its x shard (8.4 MB bf16) is loaded to SBUF once
and serves as the moving operand for BOTH the LoRA down-proj and the main
GEMM. W streams through once (50 MB bf16 per core).

All matmul inputs are bf16 (host-converted): bf16 enables the fast weight
load path (FWL, 4 elem/cycle — fp32/fp32r cannot FWL), so the per-matmul
stationary reload (~27 ns) hides under the previous matmul's 512-column
stream inside the PE's 64-deep reorder window. PSUM accumulates in fp32, so
the only precision loss is the one-time bf16 input rounding (~1.5e-3 rel).

The LoRA up-proj is fused into the main GEMM's PSUM accumulation: per output
row-block mb, after the 32 K-tile matmuls, one extra matmul with B as the
stationary operand (contracting the 128 (slot,rank) pairs of the gated
down-proj activations) lands the LoRA delta in the same PSUM bank
(start=False). A single drain then emits final y rows — no partials, no
host-side reduce.
"""

import numpy as np

# problem shape (hardcoded per harness contract)
T = 8192
HID = 4096
Q_SIZE = 4096
KV_SIZE = 1024
D = Q_SIZE + 2 * KV_SIZE  # 6144
S = 8
R = 16
NCORES = 8
P = 128

TC = T // NCORES          # 1024 tokens per core
KA = HID // P             # 32 k-tiles
MB = D // P               # 48 output row-blocks of 128
NH = TC // 512            # 2 moving n-halves of 512 tokens
# k-tiles per down-proj/x-load chunk: tiny first chunks so the first matmul
# only waits on ~0.36 MB of DMA, larger steady-state chunks
CHUNKS = [1, 1, 2, 4, 4, 4, 4, 4, 4, 4]
assert sum(CHUNKS) == KA

_CACHE = {}


def _build_nc():
    import concourse.mybir as mybir
    import concourse.tile as tile
    from concourse import bacc

    bf16 = mybir.dt.bfloat16
    f32 = mybir.dt.float32
    COPY = mybir.ActivationFunctionType.Copy

    nc = bacc.Bacc(None, target_bir_lowering=False, debug=False)

    # ---- DRAM parameters (per-core shapes; declaration order = binding order)
    x_d = nc.declare_dram_parameter("x_t", [P, KA, TC], bf16, isOutput=False)
    a_d = nc.declare_dram_parameter("a_t", [P, KA, 3, P], bf16, isOutput=False)
    w_d = nc.declare_dram_parameter("w_t", [MB, P, KA, P], bf16, isOutput=False)
    b_d = nc.declare_dram_parameter("b_t", [P, MB, P], bf16, isOutput=False)
    g_d = nc.declare_dram_parameter("gate", [P, TC], f32, isOutput=False)
    y_d = nc.declare_dram_parameter("y", [MB, P, TC], f32, isOutput=True)

    with tile.TileContext(nc) as tc:
        with tc.tile_pool(name="xres", bufs=1) as xres_pool, \
             tc.tile_pool(name="wp", bufs=3) as w_pool, \
             tc.tile_pool(name="ap", bufs=16) as a_pool, \
             tc.tile_pool(name="agp", bufs=1) as ag_pool, \
             tc.tile_pool(name="bp", bufs=1) as b_pool, \
             tc.tile_pool(name="stp", bufs=4) as st_pool, \
             tc.tile_pool(name="psum", bufs=8, space="PSUM") as ps_pool:

            # resident moving operand: x shard [p(k), kt, t], filled chunkwise
            x_res = xres_pool.tile([P, KA, TC], bf16, tag="xres")

            # ---------------- Phase A: LoRA down-proj aT = A @ x ------------
            # aT[(i,sr), t] accumulated in 6 psum banks over all 32 k-tiles,
            # chasing the chunked x/A loads on the SP (HWDGE) queue. The
            # B/gate loads ride the ACT queue (idle until outputs start), so
            # the first matmul starts after ~0.7 MB instead of ~3.5 MB.
            ps_a = [
                ps_pool.tile([P, 512], f32, tag="ps", name=f"ps_a{i}_{h}")
                for i in range(3) for h in range(2)
            ]
            a_tiles = []
            k0s = [sum(CHUNKS[:c]) for c in range(len(CHUNKS))]
            for ch, (k0, cw) in enumerate(zip(k0s, CHUNKS)):
                ksl = slice(k0, k0 + cw)
                nc.sync.dma_start(out=x_res[:, ksl, :], in_=x_d[:, ksl, :])
                a_t = a_pool.tile([P, cw, 3, P], bf16, tag="a", name=f"a{ch}")
                nc.sync.dma_start(out=a_t[:], in_=a_d[:, ksl, :, :])
                a_tiles.append(a_t)
            b_t = b_pool.tile([P, MB, P], bf16, tag="b")
            nc.scalar.dma_start(out=b_t[:], in_=b_d[:])
            gate_t = ag_pool.tile([P, TC], f32, tag="gate")
            nc.scalar.dma_start(out=gate_t[:], in_=g_d[:])
            for ch, (k0, cw) in enumerate(zip(k0s, CHUNKS)):
                a_t = a_tiles[ch]
                for kk in range(cw):
                    first = k0 + kk == 0
                    last = k0 + kk == KA - 1
                    for i in range(3):
                        for h in range(2):
                            nc.tensor.matmul(
                                ps_a[i * 2 + h][:],
                                a_t[:, kk, i, :],
                                x_res[:, k0 + kk, h * 512:(h + 1) * 512],
                                start=first, stop=last,
                            )

            # ---------------- Phase B: routing gate -------------------------
            ag = []
            for i in range(3):
                ag_t = ag_pool.tile([P, TC], bf16, tag=f"ag{i}", name=f"ag{i}")
                for h in range(2):
                    sl = slice(h * 512, (h + 1) * 512)
                    nc.vector.tensor_mul(ag_t[:, sl], ps_a[i * 2 + h][:], gate_t[:, sl])
                ag.append(ag_t)

            # ---------------- Phase C: main GEMM + fused LoRA up-proj -------
            for mb in range(MB):
                w_t = w_pool.tile([P, KA, P], bf16, tag="w", name=f"w{mb}")
                nc.sync.dma_start(out=w_t[:], in_=w_d[mb])
                i = 0 if mb < Q_SIZE // P else (1 if mb < (Q_SIZE + KV_SIZE) // P else 2)
                pss = [
                    ps_pool.tile([P, 512], f32, tag="ps", name=f"pm{mb}_{j}")
                    for j in range(NH)
                ]
                for kt in range(KA):
                    for j in range(NH):
                        nc.tensor.matmul(
                            pss[j][:],
                            w_t[:, kt, :],
                            x_res[:, kt, j * 512:(j + 1) * 512],
                            start=(kt == 0), stop=False,
                        )
                for j in range(NH):
                    nc.tensor.matmul(
                        pss[j][:],
                        b_t[:, mb, :],
                        ag[i][:, j * 512:(j + 1) * 512],
                        start=False, stop=True,
                    )
                st = st_pool.tile([P, TC], f32, tag="st", name=f"st{mb}")
                if mb < MB - 1:
                    nc.vector.tensor_copy(st[:, 0:512], pss[0][:])
                    nc.scalar.activation(st[:, 512:1024], pss[1][:], COPY)
                    nc.scalar.dma_start(out=y_d[mb, :, 0:512], in_=st[:, 0:512])
                    nc.scalar.dma_start(out=y_d[mb, :, 512:1024], in_=st[:, 512:1024])
                else:
                    # last drain is on the critical path: quarter-granular so
                    # the out-DMAs overlap the copies
                    for q in range(4):
                        sl = slice(q * 256, (q + 1) * 256)
                        eng = nc.vector if q % 2 == 0 else nc.scalar
                        if q % 2 == 0:
                            eng.tensor_copy(st[:, sl], pss[q // 2][:, q % 2 * 256:q % 2 * 256 + 256])
                        else:
                            eng.activation(st[:, sl], pss[q // 2][:, 256:512], COPY)
                        nc.scalar.dma_start(out=y_d[mb, :, sl], in_=st[:, sl])

    nc.compile()
    return nc


def _get_nc():
    if "nc" not in _CACHE:
        _CACHE["nc"] = _build_nc()
    return _CACHE["nc"]


def _prep_in_maps(x, W, lora_A, lora_B_q, lora_B_k, lora_B_v, scaling, token_to_slot):
    import ml_dtypes
    bf = ml_dtypes.bfloat16
    f = np.float32
    x = np.asarray(x, dtype=f)
    W = np.asarray(W, dtype=f)

    # x moving operand, token-sharded: [c, p(k), kt, tl]  (t = c*1024 + tl)
    x_t = np.ascontiguousarray(
        x.reshape(NCORES, TC, KA, P).transpose(0, 3, 2, 1).astype(bf))
    # main GEMM stationary (replicated): [mb, p(k), kt, dl]  (d = mb*128 + dl)
    w_t = np.ascontiguousarray(
        W.reshape(MB, P, KA, P).transpose(0, 3, 2, 1).astype(bf))
    # LoRA A stationary: [p(k), kt, i, (s r)]
    a_t = np.ascontiguousarray(
        np.asarray(lora_A, dtype=f).reshape(S, 3, R, KA, P).transpose(4, 3, 1, 0, 2)
        .reshape(P, KA, 3, S * R).astype(bf))
    # LoRA B stationary: [(s r), mb, dl]
    bq = np.asarray(lora_B_q, dtype=f).transpose(0, 2, 1).reshape(S * R, Q_SIZE)
    bk = np.asarray(lora_B_k, dtype=f).transpose(0, 2, 1).reshape(S * R, KV_SIZE)
    bv = np.asarray(lora_B_v, dtype=f).transpose(0, 2, 1).reshape(S * R, KV_SIZE)
    b_t = np.ascontiguousarray(
        np.concatenate([bq, bk, bv], axis=1).reshape(S * R, MB, P).astype(bf))
    # routing gate, expanded over ranks: [c, (s r), tl]
    slot = np.asarray(token_to_slot).reshape(NCORES, TC)
    g = (slot[:, None, :] == np.arange(S, dtype=slot.dtype)[None, :, None])
    g = g.astype(f) * np.asarray(scaling, dtype=f)[None, :, None]
    gate = np.ascontiguousarray(np.repeat(g, R, axis=1))

    in_maps = []
    for c in range(NCORES):
        in_maps.append({
            "x_t": x_t[c],
            "a_t": a_t,
            "w_t": w_t,
            "b_t": b_t,
            "gate": gate[c],
        })
    return in_maps


def _assemble(results):
    # y[c] is [mb, dl, tl] fp32 — final values for core c's token shard
    return np.ascontiguousarray(np.concatenate(
        [results[c]["y"].reshape(D, TC).T for c in range(NCORES)], axis=0))


def _run(inputs, trace=False):
    from concourse.bass_utils import run_bass_kernel_spmd
    nc = _get_nc()
    in_maps = _prep_in_maps(**inputs)
    res = run_bass_kernel_spmd(
        nc, in_maps, core_ids=list(range(NCORES)), trace=trace)
    return res


def kernel(**inputs) -> np.ndarray:
    res = _run(inputs, trace=False)
    return _assemble(res.results)


if __name__ == "__main__":
    rng = np.random.default_rng(0)
    ins = {
        "x": rng.standard_normal((T, HID)).astype(np.float32),
        "W": (rng.standard_normal((D, HID)) * 0.02).astype(np.float32),
        "lora_A": (rng.standard_normal((S, 3, R, HID)) * 0.02).astype(np.float32),
        "lora_B_q": (rng.standard_normal((S, Q_SIZE, R)) * 0.02).astype(np.float32),
        "lora_B_k": (rng.standard_normal((S, KV_SIZE, R)) * 0.02).astype(np.float32),
        "lora_B_v": (rng.standard_normal((S, KV_SIZE, R)) * 0.02).astype(np.float32),
        "scaling": rng.uniform(0.5, 2.0, S).astype(np.float32),
        "token_to_slot": rng.integers(0, S, T).astype(np.int32),
    }
    out = kernel(**ins)
    print("out", out.shape, out.dtype)


# revision 12
# speedup vs baseline: 1.4692x; 1.0020x over previous
"""LoRA QKV fused projection kernel for 8 TRN2 NeuronCores.

Reference computation (T=8192 tokens, HID=4096, D=6144 out, S=8 slots, R=16):
    y = x @ W.T
    a[t,s,i,r] = sum_h x[t,h] * lora_A[s,i,r,h]         (down-proj, all slots)
    a *= onehot(token_to_slot)[t,s] * scaling[s]         (routing gate)
    d[t, :] = concat_i( sum_{s,r} a[t,s,i,r] * B_i[s,:,r] )   (up-proj)
    out = y + d

Sharding: pure token-DP — core c owns tokens [c*1024, (c+1)*1024) with the
full hidden and output dims. Its x shard (8.4 MB bf16) is loaded to SBUF once
and serves as the moving operand for BOTH the LoRA down-proj and the main
GEMM. W streams through once (50 MB bf16 per core).

All matmul inputs are bf16 (host-converted): bf16 enables the fast weight
load path (FWL, 4 elem/cycle — fp32/fp32r cannot FWL), so the per-matmul
stationary reload (~27 ns) hides under the previous matmul's 512-column
stream inside the PE's 64-deep reorder window. PSUM accumulates in fp32, so
the only precision loss is the one-time bf16 input rounding (~1.5e-3 rel).

The LoRA up-proj is fused into the main GEMM's PSUM accumulation: per output
row-block mb, after the 32 K-tile matmuls, one extra matmul with B as the
stationary operand (contracting the 128 (slot,rank) pairs of the gated
down-proj activations) lands the LoRA delta in the same PSUM bank
(start=False). A single drain then emits final y rows — no partials, no
host-side reduce.
"""

import numpy as np

# problem shape (hardcoded per harness contract)
T = 8192
HID = 4096
Q_SIZE = 4096
KV_SIZE = 1024
D = Q_SIZE + 2 * KV_SIZE  # 6144
S = 8
R = 16
NCORES = 8
P = 128

TC = T // NCORES          # 1024 tokens per core
KA = HID // P             # 32 k-tiles
MB = D // P               # 48 output row-blocks of 128
NH = TC // 512            # 2 moving n-halves of 512 tokens
# k-tiles per down-proj/x-load chunk: tiny first chunks so the first matmul
# only waits on ~0.36 MB of DMA, larger steady-state chunks
CHUNKS = [1, 1, 2, 4, 4, 4, 4, 4, 4, 4]
assert sum(CHUNKS) == KA

_CACHE = {}


def _build_nc():
    import concourse.mybir as mybir
    import concourse.tile as tile
    from concourse import bacc

    bf16 = mybir.dt.bfloat16
    f32 = mybir.dt.float32
    COPY = mybir.ActivationFunctionType.Copy

    nc = bacc.Bacc(None, target_bir_lowering=False, debug=False)

    # ---- DRAM parameters (per-core shapes; declaration order = binding order)
    x_d = nc.declare_dram_parameter("x_t", [P, KA, TC], bf16, isOutput=False)
    a_d = nc.declare_dram_parameter("a_t", [P, KA, 3, P], bf16, isOutput=False)
    w_d = nc.declare_dram_parameter("w_t", [MB, P, KA, P], bf16, isOutput=False)
    b_d = nc.declare_dram_parameter("b_t", [P, MB, P], bf16, isOutput=False)
    g_d = nc.declare_dram_parameter("gate", [P, TC], f32, isOutput=False)
    y_d = nc.declare_dram_parameter("y", [MB, P, TC], f32, isOutput=True)

    with tile.TileContext(nc) as tc:
        with tc.tile_pool(name="xres", bufs=1) as xres_pool, \
             tc.tile_pool(name="wp", bufs=3) as w_pool, \
             tc.tile_pool(name="ap", bufs=16) as a_pool, \
             tc.tile_pool(name="agp", bufs=1) as ag_pool, \
             tc.tile_pool(name="bp", bufs=1) as b_pool, \
             tc.tile_pool(name="stp", bufs=4) as st_pool, \
             tc.tile_pool(name="psum", bufs=8, space="PSUM") as ps_pool:

            # resident moving operand: x shard [p(k), kt, t], filled chunkwise
            x_res = xres_pool.tile([P, KA, TC], bf16, tag="xres")

            # ---------------- Phase A: LoRA down-proj aT = A @ x ------------
            # aT[(i,sr), t] accumulated in 6 psum banks over all 32 k-tiles,
            # chasing the chunked x/A loads on the SP (HWDGE) queue. The
            # B/gate loads ride the ACT queue (idle until outputs start), so
            # the first matmul starts after ~0.7 MB instead of ~3.5 MB.
            ps_a = [
                ps_pool.tile([P, 512], f32, tag="ps", name=f"ps_a{i}_{h}")
                for i in range(3) for h in range(2)
            ]
            # x (and later w) on the SP ring; a/gate/b on the ACT ring so the
            # first matmul's two inputs stream in parallel
            a_tiles = []
            k0s = [sum(CHUNKS[:c]) for c in range(len(CHUNKS))]
            for ch, (k0, cw) in enumerate(zip(k0s, CHUNKS)):
                ksl = slice(k0, k0 + cw)
                nc.sync.dma_start(out=x_res[:, ksl, :], in_=x_d[:, ksl, :])
                a_t = a_pool.tile([P, cw, 3, P], bf16, tag="a", name=f"a{ch}")
                nc.scalar.dma_start(out=a_t[:], in_=a_d[:, ksl, :, :])
                a_tiles.append(a_t)
            gate_t = ag_pool.tile([P, TC], f32, tag="gate")
            nc.scalar.dma_start(out=gate_t[:], in_=g_d[:])
            b_t = b_pool.tile([P, MB, P], bf16, tag="b")
            nc.scalar.dma_start(out=b_t[:], in_=b_d[:])
            for ch, (k0, cw) in enumerate(zip(k0s, CHUNKS)):
                a_t = a_tiles[ch]
                for kk in range(cw):
                    first = k0 + kk == 0
                    last = k0 + kk == KA - 1
                    for i in range(3):
                        for h in range(2):
                            nc.tensor.matmul(
                                ps_a[i * 2 + h][:],
                                a_t[:, kk, i, :],
                                x_res[:, k0 + kk, h * 512:(h + 1) * 512],
                                start=first, stop=last,
                            )

            # ---------------- Phase B: routing gate -------------------------
            ag = []
            for i in range(3):
                ag_t = ag_pool.tile([P, TC], bf16, tag=f"ag{i}", name=f"ag{i}")
                for h in range(2):
                    sl = slice(h * 512, (h + 1) * 512)
                    nc.vector.tensor_mul(ag_t[:, sl], ps_a[i * 2 + h][:], gate_t[:, sl])
                ag.append(ag_t)

            # ---------------- Phase C: main GEMM + fused LoRA up-proj -------
            for mb in range(MB):
                w_t = w_pool.tile([P, KA, P], bf16, tag="w", name=f"w{mb}")
                nc.sync.dma_start(out=w_t[:], in_=w_d[mb])
                i = 0 if mb < Q_SIZE // P else (1 if mb < (Q_SIZE + KV_SIZE) // P else 2)
                pss = [
                    ps_pool.tile([P, 512], f32, tag="ps", name=f"pm{mb}_{j}")
                    for j in range(NH)
                ]
                for kt in range(KA):
                    for j in range(NH):
                        nc.tensor.matmul(
                            pss[j][:],
                            w_t[:, kt, :],
                            x_res[:, kt, j * 512:(j + 1) * 512],
                            start=(kt == 0), stop=False,
                        )
                for j in range(NH):
                    nc.tensor.matmul(
                        pss[j][:],
                        b_t[:, mb, :],
                        ag[i][:, j * 512:(j + 1) * 512],
                        start=False, stop=True,
                    )
                st = st_pool.tile([P, TC], f32, tag="st", name=f"st{mb}")
                nc.vector.tensor_copy(st[:, 0:512], pss[0][:])
                nc.scalar.activation(st[:, 512:1024], pss[1][:], COPY)
                nc.scalar.dma_start(out=y_d[mb, :, 0:512], in_=st[:, 0:512])
                nc.scalar.dma_start(out=y_d[mb, :, 512:1024], in_=st[:, 512:1024])

    nc.compile()
    return nc


def _get_nc():
    if "nc" not in _CACHE:
        _CACHE["nc"] = _build_nc()
    return _CACHE["nc"]


def _prep_in_maps(x, W, lora_A, lora_B_q, lora_B_k, lora_B_v, scaling, token_to_slot):
    import ml_dtypes
    bf = ml_dtypes.bfloat16
    f = np.float32
    x = np.asarray(x, dtype=f)
    W = np.asarray(W, dtype=f)

    # x moving operand, token-sharded: [c, p(k), kt, tl]  (t = c*1024 + tl)
    x_t = np.ascontiguousarray(
        x.reshape(NCORES, TC, KA, P).transpose(0, 3, 2, 1).astype(bf))
    # main GEMM stationary (replicated): [mb, p(k), kt, dl]  (d = mb*128 + dl)
    w_t = np.ascontiguousarray(
        W.reshape(MB, P, KA, P).transpose(0, 3, 2, 1).astype(bf))
    # LoRA A stationary: [p(k), kt, i, (s r)]
    a_t = np.ascontiguousarray(
        np.asarray(lora_A, dtype=f).reshape(S, 3, R, KA, P).transpose(4, 3, 1, 0, 2)
        .reshape(P, KA, 3, S * R).astype(bf))
    # LoRA B stationary: [(s r), mb, dl]
    bq = np.asarray(lora_B_q, dtype=f).transpose(0, 2, 1).reshape(S * R, Q_SIZE)
    bk = np.asarray(lora_B_k, dtype=f).transpose(0, 2, 1).reshape(S * R, KV_SIZE)
    bv = np.asarray(lora_B_v, dtype=f).transpose(0, 2, 1).reshape(S * R, KV_SIZE)
    b_t = np.ascontiguousarray(
        np.concatenate([bq, bk, bv], axis=1).reshape(S * R, MB, P).astype(bf))
    # routing gate, expanded over ranks: [c, (s r), tl]
    slot = np.asarray(token_to_slot).reshape(NCORES, TC)
    g = (slot[:, None, :] == np.arange(S, dtype=slot.dtype)[None, :, None])
    g = g.astype(f) * np.asarray(scaling, dtype=f)[None, :, None]
    gate = np.ascontiguousarray(np.repeat(g, R, axis=1))

    in_maps = []
    for c in range(NCORES):
        in_maps.append({
            "x_t": x_t[c],
            "a_t": a_t,
            "w_t": w_t,
            "b_t": b_t,
            "gate": gate[c],
        })
    return in_maps


def _assemble(results):
    # y[c] is [mb, dl, tl] fp32 — final values for core c's token shard
    return np.ascontiguousarray(np.concatenate(
        [results[c]["y"].reshape(D, TC).T for c in range(NCORES)], axis=0))


def _run(inputs, trace=False):
    from concourse.bass_utils import run_bass_kernel_spmd
    nc = _get_nc()
    in_maps = _prep_in_maps(**inputs)
    res = run_bass_kernel_spmd(
        nc, in_maps, core_ids=list(range(NCORES)), trace=trace)
    return res


def kernel(**inputs) -> np.ndarray:
    res = _run(inputs, trace=False)
    return _assemble(res.results)


if __name__ == "__main__":
    rng = np.random.default_rng(0)
    ins = {
        "x": rng.standard_normal((T, HID)).astype(np.float32),
        "W": (rng.standard_normal((D, HID)) * 0.02).astype(np.float32),
        "lora_A": (rng.standard_normal((S, 3, R, HID)) * 0.02).astype(np.float32),
        "lora_B_q": (rng.standard_normal((S, Q_SIZE, R)) * 0.02).astype(np.float32),
        "lora_B_k": (rng.standard_normal((S, KV_SIZE, R)) * 0.02).astype(np.float32),
        "lora_B_v": (rng.standard_normal((S, KV_SIZE, R)) * 0.02).astype(np.float32),
        "scaling": rng.uniform(0.5, 2.0, S).astype(np.float32),
        "token_to_slot": rng.integers(0, S, T).astype(np.int32),
    }
    out = kernel(**ins)
    print("out", out.shape, out.dtype)
